# revision 92
# baseline (speedup 1.0000x reference)
"""Cumulative-probability head on 8 Trainium2 NeuronCores.

out[b, j] = sum_{i<=j} relu(x @ W_h^T + b_h)[b, i] + (x @ W_base^T + b_base)[b]

Data-parallel: x sharded along batch (1024 rows/core), weights replicated.

Per-core strategy (fp8 DoubleRow):
  - x and W quantized host-side to fp8-e4m3 with power-of-2 scales Sx=16,
    Sw=512; hazard matmuls run DoubleRowSwInterleave (2 fp8 MACs/cell/cyc),
    256 contraction per chunk, 8 chunks. 8 b-tiles of 128 rows.
  - Column split: each chunk does TWO N=258 matmuls per tile — group A
    covers hazard cols 0:256 plus the base column (col 256) riding the
    same PSUM bank; group B covers cols 256:512 (+2 pad). This removes
    the separate N=2 base matmuls (a ~60ns dispatch floor each).
  - Bias: the bank is *seeded* with S*b by non-PE engines (DVE for the
    wave-0 tiles, scalar/DVE split for wave-1) and the chunk matmuls
    accumulate with start=False — no PE opener/closer matmuls at all.
    has_written bits are pre-set by full-bank warm-up dummies; start=True
    is never issued on a bank after that. Bias rows (prescaled S/128,
    replicated over 128 partitions) ride the tail of chunk 0's x DMA.
  - HAM clock gate: ~4.3us of full-K bf16 dummies at kernel entry (read
    an uninitialized SBUF tile, so they issue immediately) warm the PE
    clock to 2.4GHz before the chunk stream starts; filler dummies in
    wave-0's tail keep the monitor fed through DMA waits (K=1 matmuls do
    NOT count as activity; >~1us idle restarts the ramp).
  - DMAs: host-permuted DRAM layouts give per-partition-contiguous rows
    (0.75-2KB packets; ring throughput scales with packet size). Pieces
    are deadline-ordered and round-robined over the sync/scalar/gpsimd
    rings; which ring spins up first is run-variable, so chunks 0-3 ride
    fine-grained single-chunk DMAs and 4-7 ride pair DMAs.
  - Wave 0 (tiles 0-2): chunk-outer, paced by the input stream; its three
    early drains start the DVE scan chain ~4us sooner than a 4-tile wave.
    Wave 1 (tiles 3-7): b-outer on resident data; t7 runs group A's 8
    chunks to a stop before group B so its scan overlaps B's matmuls, and
    its drain is 3-way split so the last out DMA is only 16KB.
  - Drain per tile: baset = Ident(psA[:,256])/S on ScalarE, Relu/S of
    psA/psB into bf16 haz, DVE tensor_tensor_scan (fp32 state) seeded
    with baset, bf16 out DMAs on whichever ring is still warm.
  - PSUM: 8 banks exactly = 4 tiles in flight x (psA, psB).
"""

import numpy as np
import ml_dtypes

import concourse.bass as bass
import concourse.tile as tile
from concourse import bacc, mybir
from concourse.bass_utils import run_bass_kernel_spmd

B, D, T = 8192, 2048, 512
NCORES = 8
BLOC = B // NCORES            # 1024 rows per core
NT = BLOC // 128              # 8 b-tiles per core
NCH = D // 256                # 8 contraction chunks (256 = 128 x 2 doublerow)
NG = 258                      # matmul N per column group (256 hazard + 2)
TP = 2 * NG                   # 516 = padded wt width per (chunk, slot)
SX = 16.0                     # x fp8 scale
SW = 512.0                    # W fp8 scale
S = SX * SW

F32 = mybir.dt.float32
BF16 = mybir.dt.bfloat16
F8 = mybir.dt.float8e4

F8NP = ml_dtypes.float8_e4m3
BF16NP = ml_dtypes.bfloat16

DR = mybir.MatmulPerfMode.DoubleRowSwInterleave
Relu = mybir.ActivationFunctionType.Relu
Ident = mybir.ActivationFunctionType.Identity


def _build_program():
    nc = bacc.Bacc("TRN2", target_bir_lowering=False, debug=False)

    # DRAM layouts (host-permuted for contiguous per-partition DMA rows):
    #  wt:  [128 p][8 c][2 slot][516 col]  (col = [h0:256|base|0 | h256:512|0|0])
    #  x0:  [4 pair][128 p][2 cc][4 bl][256 swi]   wave-0 rows 0:512
    #  x1:  [4 tile][128 p][8 c][256 swi]          wave-1 rows 512:1024
    #  b8:  [1][516]  = S/16 * [b_h[0:256], b_base, 0, b_h[256:512], 0, 0]
    wt_d = nc.dram_tensor("wt", [128, NCH, 2, TP], F8, kind="ExternalInput")
    # x0 chunk 0 rides its own tensor with the (pre-replicated, S/128-scaled)
    # bias rows in its tail 516B/partition: openers share chunk 0's
    # dependency and need no separate bias DMA.
    xb_d = nc.dram_tensor("xb", [128, 768 + TP], F8, kind="ExternalInput")
    x0_d = nc.dram_tensor("x0", [4, 128, 768], F8, kind="ExternalInput")
    x0p_d = nc.dram_tensor("x0p", [2, 128, 1536], F8, kind="ExternalInput")
    x1_d = nc.dram_tensor("x1", [5, 128, 2048], F8, kind="ExternalInput")
    out_d = nc.dram_tensor("out", [BLOC, T], BF16, kind="ExternalOutput")

    with tile.TileContext(nc) as tc:
        with (
            tc.tile_pool(name="consts", bufs=1) as consts,
            tc.tile_pool(name="wt", bufs=1) as wtp,
            tc.tile_pool(name="xt", bufs=1) as xtp,
            tc.tile_pool(name="haz", bufs=4) as hazp,
            tc.tile_pool(name="outp", bufs=4) as outp,
            tc.tile_pool(name="ps", bufs=8, space="PSUM") as psp,
        ):
            # Small consts. zeros memset first on the DVE queue: the HAM
            # warm-up dummies read it, and they must be full-K (128
            # partitions) matmuls — K=1 matmuls don't register as PE
            # activity for the clock-gate monitor.
            zeros = consts.tile([128, T], BF16, tag="zeros")
            nc.vector.memset(zeros, 0.0)
            # Deliberately uninitialized raw SBUF tensor: HAM warm-up dummies
            # read it so they can issue at tile-entry with no memset
            # dependency; their PSUM output is never read.
            garbage = nc.alloc_sbuf_tensor("ham_garbage", [128, T], BF16)


            wt_tiles = [
                wtp.tile([128, 2, TP], F8, tag=f"w{i}", name=f"w{i}")
                for i in range(4)
            ]
            x0_tiles = [
                xtp.tile([128, 768 + TP] if i == 0 else [128, 768], F8,
                         tag=f"x0_{i}", name=f"x0_{i}")
                for i in range(4)
            ]
            x1_tiles = [
                xtp.tile([128, 2048], F8, tag=f"x1_{i}", name=f"x1_{i}")
                for i in range(5)
            ]

            # Input DMA issue order per ring (in-queue order = issue order).
            # Measured: rings spin up ~0.8/1.8/2.7us after their first issue,
            # in first-issue order, and share ~430GB/s. So the critical lead
            # pieces (bias row, w chunk 0, x0 chunk 0) all ride sync, and
            # chunks 0/1 are single-chunk DMAs for fine-grained pacing.
            # ~1.03MB per ring, strictly deadline-ordered: rings share
            # ~320-430GB/s once all three stream, and which ring spins up
            # first is run-variable — so chunk 0's pieces are split small
            # across ALL rings and every ring leads with its earliest-needed
            # bytes.
            # Pieces in strict deadline order, round-robined over the three
            # rings. Ring throughput is roughly proportional to packet size
            # and any ring can randomly be the slow starter, so pieces are
            # uniform-sized (768-2048B rows) and no chunk has both its w and
            # x piece on the same ring.
            pieces = []
            for c in range(4):
                pieces.append((wt_tiles[c], wt_d[:, c]))
                pieces.append((x0_tiles[c], xb_d[:, :] if c == 0 else x0_d[c]))
            # Chunks 4-7 as pair transfers: fewer completions to wait on and
            # larger packets (ring share is roughly proportional to packet
            # size), which tightens the wave-0 tail against ring-speed luck.
            w45 = wtp.tile([128, 2, 2, TP], F8, tag="w45p", name="w45p")
            w67 = wtp.tile([128, 2, 2, TP], F8, tag="w67p", name="w67p")
            x45 = xtp.tile([128, 1536], F8, tag="x45p", name="x45p")
            x67 = xtp.tile([128, 1536], F8, tag="x67p", name="x67p")
            pieces += [
                (w45, wt_d[:, 4:6]),
                (x45, x0p_d[0]),
                (w67, wt_d[:, 6:8]),
                (x67, x0p_d[1]),
            ]
            for i in range(5):
                pieces.append((x1_tiles[i], x1_d[i]))
            rings3 = [nc.sync, nc.scalar, nc.gpsimd]
            for i, (dst, src) in enumerate(pieces):
                rings3[i % 3].dma_start(out=dst, in_=src)

            def rhs_w(c, g):
                # [128, 2 slot, 258] slice of chunk c's weight tile.
                if c < 4:
                    return wt_tiles[c][:, :, NG * g : NG * (g + 1)]
                wp = w45 if c < 6 else w67
                return wp[:, c % 2, :, NG * g : NG * (g + 1)]

            def lhsT_w0(c, bl):
                # SWI stationary: block q = 2*(127-m) + i, at 256*bl.
                if c < 4:
                    sl = x0_tiles[c][:, 256 * bl : 256 * (bl + 1)]
                else:
                    xp = x45 if c < 6 else x67
                    o = 768 * (c % 2) + 256 * bl
                    sl = xp[:, o : o + 256]
                return bass.AP(
                    tensor=sl.tensor,
                    offset=sl.offset,
                    ap=[list(sl.ap[0]), [1, 2], [2, 128]],
                )

            def lhsT_w1(t, c):
                sl = x1_tiles[t][:, 256 * c : 256 * (c + 1)]
                return bass.AP(
                    tensor=sl.tensor,
                    offset=sl.offset,
                    ap=[list(sl.ap[0]), [1, 2], [2, 128]],
                )

            # Bias seeding: instead of PE opener matmuls (which cost ~2.2us
            # of tensor-engine stream), idle engines write S*b straight into
            # PSUM (seed = 128 * xb_bias_fp8). The chunk matmuls then
            # accumulate with start=False; their banks' has_written bits are
            # pre-set by the full-bank warm-up dummies, and no start=True is
            # ever issued on a bank again.
            def seed_dve(ps, g):
                nc.vector.tensor_scalar_mul(
                    ps[:, 0:NG],
                    x0_tiles[0][:, 768 + NG * g : 768 + NG * (g + 1)],
                    128.0,
                )

            def seed_act(ps, g):
                nc.scalar.activation(
                    out=ps[:, 0:NG],
                    in_=x0_tiles[0][:, 768 + NG * g : 768 + NG * (g + 1)],
                    func=Ident,
                    scale=128.0,
                )

            def chunk_mm(ps, lhsT, c, g, stop):
                nc.tensor.matmul(
                    ps[:, 0:NG],
                    lhsT,
                    rhs_w(c, g),
                    start=False,
                    stop=stop,
                    perf_mode=DR,
                    skip_group_check=True,
                )

            # PSUM tiles: 8 banks; tile t uses (psA, psB); pool rotation
            # reuses wave-0 banks for wave-1 after their drains.
            def ps_pair(t):
                a = psp.tile([128, 512], F32, tag="ps", name=f"psA{t}")
                b = psp.tile([128, 512], F32, tag="ps", name=f"psB{t}")
                return a, b

            # Out rings: early tiles can ride the slow software ring, but the
            # final tiles must ride fast rings or their DMA drain extends the
            # epilogue. Scalar gets only t7's tail half (nothing queued after
            # its last ACT, so the in-order scan wait is harmless there).
            out_ring_of = {0: nc.gpsimd, 1: nc.sync, 2: nc.gpsimd, 3: nc.sync,
                           4: nc.gpsimd, 5: nc.sync, 6: nc.sync}
            # t7's first two outs on sync (still streaming); the final tiny
            # one issues from the scalar queue, which has nothing after
            # t7's ACTs, so its issue overlaps sync's.
            t7_rings = [nc.sync, nc.scalar]

            def drain(t, psA, psB, split_dma=1):
                baset = hazp.tile([128, 1], BF16, tag="base", name=f"base{t}")
                nc.scalar.activation(
                    out=baset, in_=psA[:, 256:257], func=Ident, scale=1.0 / S
                )
                haz = hazp.tile([128, T], BF16, tag="haz", name=f"haz{t}")
                nc.scalar.activation(
                    out=haz[:, 0:256], in_=psA[:, 0:256], func=Relu, scale=1.0 / S
                )
                nc.scalar.activation(
                    out=haz[:, 256:512], in_=psB[:, 0:256], func=Relu, scale=1.0 / S
                )
                cum = outp.tile([128, T], BF16, tag="cum", name=f"cum{t}")
                # split_dma: list of column boundaries. 2 = halves (scan of
                # [0:256] starts after relu-A alone); 3 adds a small final
                # segment to shrink the last out DMA gating the epilogue.
                bounds = {1: [0, T], 2: [0, 256, T], 3: [0, 256, 448, T]}[split_dma]
                for q in range(len(bounds) - 1):
                    lo, hi = bounds[q], bounds[q + 1]
                    nc.vector.tensor_tensor_scan(
                        out=cum[:, lo:hi],
                        data0=haz[:, lo:hi],
                        data1=zeros[:, lo:hi],
                        initial=baset if q == 0 else cum[:, lo - 1 : lo],
                        op0=mybir.AluOpType.add,
                        op1=mybir.AluOpType.add,
                    )
                    ring = t7_rings[q == 2] if t == 7 else out_ring_of[t]
                    ring.dma_start(
                        out=out_d[128 * t : 128 * (t + 1), lo:hi], in_=cum[:, lo:hi]
                    )

            # ---- wave 0: tiles 0-2, pair-outer (first drains release the
            # DVE scan chain early); all 8 PSUM pairs up front so pool
            # rotation maps tile t+4's pair onto tile t's banks ----
            pairs = [ps_pair(t) for t in range(8)]
            psw0 = pairs[0:3]
            ps3A, ps3B = pairs[3]

            # HAM warm-up: full-K bf16 dummies (~427ns each cold), cycling
            # through every PSUM bank so each bank's has_written bits are
            # set before its start=False accumulation begins.
            def dummy(ps):
                nc.tensor.matmul(
                    ps[:], garbage[:, 0:128], garbage[:, 0:T], start=True, stop=True
                )
            banks = [p for pr in pairs[0:4] for p in pr]
            for i in range(9):
                dummy(banks[i % 8])
            # Seeds for wave-0 tiles + t3's psA on the (idle) DVE; t3's psB
            # is seeded later, after the filler dummies that target it.
            for t in range(3):
                seed_dve(psw0[t][0], 0)
                seed_dve(psw0[t][1], 1)
            seed_dve(ps3A, 0)
            for c in range(NCH):
                for t in range(3):
                    lh = lhsT_w0(c, t)
                    chunk_mm(psw0[t][0], lh, c, 0, c == NCH - 1)
                    chunk_mm(psw0[t][1], lh, c, 1, c == NCH - 1)
                # Filler dummies keep the HAM fed through clustered DMA
                # waits in wave-0's tail (which otherwise re-throttle the
                # PE clock); they target t3's psB, which is seeded after.
                if 2 <= c <= 6:
                    dummy(ps3B)
                if c == 6:
                    seed_dve(ps3B, 1)
            # Wave-1 bank seeds: A-half on scalar, B-half on DVE, emitted
            # inside the drain chain so the two drain engines stay balanced
            # (~1.5us/tile each) instead of scalar pacing at ~2.2.
            for t in range(3):
                drain(t, psw0[t][0], psw0[t][1], split_dma=2)
                seed_act(pairs[t + 4][0], 0)
                seed_dve(pairs[t + 4][1], 1)

            # ---- wave 1: tiles 3-7, b-outer on resident data ----
            for t in range(3, 8):
                psA, psB = pairs[t]
                if t < 7:
                    for c in range(NCH):
                        lh = lhsT_w1(t - 3, c)
                        chunk_mm(psA, lh, c, 0, c == NCH - 1)
                        chunk_mm(psB, lh, c, 1, c == NCH - 1)
                    drain(t, psA, psB)
                    if t == 3:
                        seed_act(pairs[7][0], 0)
                        seed_dve(pairs[7][1], 1)
                else:
                    # t7: finish group A first so its drain overlaps B's mms.
                    for c in range(NCH):
                        chunk_mm(psA, lhsT_w1(4, c), c, 0, c == NCH - 1)
                    for c in range(NCH):
                        chunk_mm(psB, lhsT_w1(4, c), c, 1, c == NCH - 1)
                    drain(t, psA, psB, split_dma=3)

    nc.compile()
    return nc


_NC_CACHE = None


def prep_in_maps(x, W_hazard, b_hazard, W_base, b_base):
    x = np.asarray(x, np.float32)
    Wh = np.asarray(W_hazard, np.float32)
    bh = np.asarray(b_hazard, np.float32)
    Wb = np.asarray(W_base, np.float32).reshape(D)
    bb = np.asarray(b_base, np.float32).reshape(1)

    # Column layout per (chunk, slot): [h0:256 | base | 0 | h256:512 | 0 | 0]
    wcols = np.zeros((D, TP), np.float32)
    wcols[:, 0:256] = Wh[0:256].T * SW
    wcols[:, 256] = Wb * SW
    wcols[:, NG : NG + 256] = Wh[256:512].T * SW
    np.clip(wcols, -240.0, 240.0, out=wcols)
    w8 = wcols.astype(F8NP)  # [2048 k, 516]
    # k = 256*c + 2*p + s  ->  [p][c][s][516]
    wt = np.ascontiguousarray(
        w8.reshape(NCH, 128, 2, TP).transpose(1, 0, 2, 3)
    )  # [128, 8, 2, 516]

    b8 = np.zeros((1, TP), np.float32)
    b8[0, 0:256] = bh[0:256] * (S / 128.0)
    b8[0, 256] = bb[0] * (S / 128.0)
    b8[0, NG : NG + 256] = bh[256:512] * (S / 128.0)
    np.clip(b8, -240.0, 240.0, out=b8)
    b8 = b8.astype(F8NP)

    x8 = np.clip(x * SX, -240.0, 240.0).astype(F8NP)  # [B, D]
    in_maps = []
    for i in range(NCORES):
        xs = x8[BLOC * i : BLOC * (i + 1)]  # [1024, D]
        # [tile 8, m 128, c 8, p 128, i 2], m reversed for SWI blocks.
        Y = xs.reshape(NT, 128, NCH, 128, 2)[:, ::-1, :, :, :]
        # wave 0 (tiles 0-2): chunks 0-3 as [4, 128, 768], chunks 4-7 as
        # pairs [2, 128, 1536]; chunk 0 ships separately with the bias rows
        # replicated into every partition's tail.
        Y0 = Y[0:3]  # [bl 3, m_r, c, p, i]
        x0all = Y0.transpose(2, 3, 0, 1, 4).reshape(NCH, 128, 768)
        x0 = np.ascontiguousarray(x0all[0:4])
        x0p = np.ascontiguousarray(
            x0all[4:8].reshape(2, 2, 128, 768).transpose(0, 2, 1, 3).reshape(2, 128, 1536)
        )
        xb = np.zeros((128, 768 + TP), F8NP)
        xb[:, 0:768] = x0all[0]
        xb[:, 768:] = b8[0]
        # wave 1 (tiles 3-7): [tile][p][c][ (127-m, i) ] -> [5, 128, 2048]
        Y1 = Y[3:8]  # [t, m_r, c, p, i]
        x1 = np.ascontiguousarray(
            Y1.transpose(0, 3, 2, 1, 4).reshape(5, 128, 2048)
        )
        in_maps.append({"xb": xb, "x0": x0, "x0p": x0p, "x1": x1, "wt": wt})
    return in_maps


def kernel(x, W_hazard, b_hazard, W_base, b_base):
    global _NC_CACHE
    if _NC_CACHE is None:
        _NC_CACHE = _build_program()
    in_maps = prep_in_maps(x, W_hazard, b_hazard, W_base, b_base)
    res = run_bass_kernel_spmd(_NC_CACHE, in_maps, list(range(NCORES)))
    return np.concatenate(
        [res.results[i]["out"].astype(np.float32) for i in range(NCORES)], axis=0
    )
